# revision 14
# baseline (speedup 1.0000x reference)
"""Trainium2 Bass kernel for AttentionWithRoPE, head-sharded across 8 NeuronCores.

Reference computation (fp32):
    q = (x @ Wq) * Dh^-0.5, rope'd; k = (x @ Wk), rope'd; v = x @ Wv
    out = softmax(q k^T * Dh^-0.5) v ;  final = out @ Wo + bo

Sharding: tensor-parallel over heads. Each core owns 2 of 16 heads: it gets
the full x (pre-transposed to [D, B*N], fp8), its column slices of Wq/Wk/Wv
(fp8, scaled by WS=32 to stay in e4m3 normal range), its row slice of Wo, and
returns a partial [B*N, D] output that the host sums over cores (+ bo).

Device layout choices:
  - Q^T/K^T are produced directly in [Dh, n] layout (D-contraction with x^T as
    the moving operand), so attention scores can be computed transposed
    (S^T[m, n], K stationary / Q moving) and the exp'd probabilities feed the
    attn@V matmul as the moving operand with V (natural [m, Dh] layout) as
    stationary -- no on-chip transposes anywhere.
  - Projections run as fp8 DoubleRow matmuls (2 K-tiles per pass, ~7x the
    fp16 rate on real HW). Wq/Wk use a hi+lo fp8 split of the weights to
    recover most of the weight-quantization error; Wv is single fp8.
  - RoPE pairs (even, odd) are separated by permuting the columns of Wq/Wk
    per head on the host so the (real, imag) partners sit 16 partitions apart
    within a 32-partition quadrant: the partner swap is a DVE stream_shuffle,
    and RoPE becomes shuffle + 3 vector ops with host-prepared factor tensors
    A (rr replicated) and B (+-ri). Scores are permutation-invariant since q
    and k share the permutation; the 1/Dh score scale and the 1/WS fp8
    weight-scale compensation are folded into the rope factors.
  - attn@V runs in fp8 via the mean-extraction trick: P = 1 + P' with
    P' = exp(scores) - 1 (computed fp16 -> fp8 on DVE, values ~ +-0.4 so fp8
    quantization noise is ~50x smaller than quantizing P itself). The matmul
    contribution of the "1" is sum_m v[m, :], which the host computes exactly
    (rank-1: (sum_m x) @ Wv) and the kernel deposits into the PSUM
    accumulation with a K=1 matmul. V is stored fp8 (its quantization error
    also only rides on the small P' term). Softmax denominators similarly:
    den = 2048 + colsum(P'), with the colsum computed on the PE as a
    DoubleRow ones-matmul.
  - All remaining matmul operands fp16, accumulation in fp32 PSUM. Partial
    outputs returned fp16.
"""

import os
import sys

for _p in ("/opt/trn_rl_repo", "/root/.axon_site/_ro/trn_rl_repo"):
    if os.path.isdir(_p) and _p not in sys.path:
        sys.path.insert(0, _p)

import numpy as np
from contextlib import ExitStack

import concourse.bass as bass
import concourse.bacc as bacc
import concourse.tile as tile
from concourse import mybir
from concourse.bass_utils import run_bass_kernel_spmd

F16 = mybir.dt.float16
F32 = mybir.dt.float32
F8 = mybir.dt.float8e4
AF = mybir.ActivationFunctionType
DR = mybir.MatmulPerfMode.DoubleRow
WS = 32.0  # fp8 weight scale (W*0.02 would sit in e4m3 subnormal range)

N_CORES = 8
B, N, D, H, Dh = 2, 2048, 2048, 16, 128
HL = H // N_CORES          # heads per core
DHL = HL * Dh              # 256 local head dims
BN = B * N                 # 4096
DCH = D // 128             # 16 contraction chunks
NBLK = BN // 512           # 8 projection column blocks
MCH = N // 128             # 16 key chunks per sequence
NCK = N // 512             # 4 query chunks per sequence

_CACHE = {}
_PHASE_MARKS = {}


def _build_nc(loop_n=1, p1_only=False, p2_mode="full"):
    nc = bacc.Bacc(trn_type="TRN2", target_bir_lowering=False, debug=False)

    x8_d = nc.dram_tensor("x8", [D, BN], F8, kind="ExternalInput")
    wq_d = nc.dram_tensor("wq", [D, DHL], F8, kind="ExternalInput")
    wql_d = nc.dram_tensor("wql", [D, DHL], F8, kind="ExternalInput")
    wk_d = nc.dram_tensor("wk", [D, DHL], F8, kind="ExternalInput")
    wkl_d = nc.dram_tensor("wkl", [D, DHL], F8, kind="ExternalInput")
    wv_d = nc.dram_tensor("wv", [D, DHL], F8, kind="ExternalInput")
    wo_d = nc.dram_tensor("wo", [DHL, D], F16, kind="ExternalInput")
    sv_d = nc.dram_tensor("sv", [1, B * DHL], F16, kind="ExternalInput")
    rope_d = nc.dram_tensor("rope", [2 * B * 2, 128, N], F16, kind="ExternalInput")
    out_d = nc.dram_tensor("out", [BN, D], F16, kind="ExternalOutput")

    x8_v = x8_d.ap().rearrange("(c p) n -> p c n", p=128)       # [128, 16, 4096]
    w_views = {
        "wq": wq_d.ap().rearrange("(c p) m -> p c m", p=128),   # [128, 16, 256]
        "wql": wql_d.ap().rearrange("(c p) m -> p c m", p=128),
        "wk": wk_d.ap().rearrange("(c p) m -> p c m", p=128),
        "wkl": wkl_d.ap().rearrange("(c p) m -> p c m", p=128),
        "wv": wv_d.ap().rearrange("(c p) m -> p c m", p=128),
    }
    wo_v = wo_d.ap().rearrange("(j p) d -> p j d", p=128)       # [128, 2, 2048]
    rope_v = rope_d.ap()                                        # [8, 128, 2048]
    out_v = out_d.ap().rearrange("(cb p) d -> cb p d", p=128)   # [32, 128, 2048]

    with tile.TileContext(nc) as tc:
        with ExitStack() as ctx:
            consts = ctx.enter_context(tc.tile_pool(name="consts", bufs=1))
            qtkt = ctx.enter_context(tc.tile_pool(name="qtkt", bufs=1))
            vres = ctx.enter_context(tc.tile_pool(name="vres", bufs=1))
            xin = ctx.enter_context(tc.tile_pool(name="xin", bufs=2))
            ropein = ctx.enter_context(tc.tile_pool(name="ropein", bufs=2))
            tmps = ctx.enter_context(tc.tile_pool(name="tmps", bufs=3))
            ptile = ctx.enter_context(tc.tile_pool(name="ptile", bufs=4))
            p8tile = ctx.enter_context(tc.tile_pool(name="p8tile", bufs=12))
            smalls = ctx.enter_context(tc.tile_pool(name="smalls", bufs=2))
            rbcp = ctx.enter_context(tc.tile_pool(name="rbcp", bufs=2))
            otbuf = ctx.enter_context(tc.tile_pool(name="otbuf", bufs=3))
            obuf = ctx.enter_context(tc.tile_pool(name="obuf", bufs=4))

            psa = ctx.enter_context(tc.tile_pool(name="psa", bufs=2, space="PSUM"))
            psb = ctx.enter_context(tc.tile_pool(name="psb", bufs=3, space="PSUM"))
            psc = ctx.enter_context(tc.tile_pool(name="psc", bufs=1, space="PSUM"))

            # ---- resident weights / constants ----
            w_sb = {}
            for wname in w_views:
                w_sb[wname] = consts.tile([128, DCH, DHL], F8, name=wname)

            def _load_w(wname):
                for dq in range(2):
                    nc.sync.dma_start(
                        w_sb[wname][:, dq * 8:(dq + 1) * 8, :],
                        w_views[wname][:, dq * 8:(dq + 1) * 8, :],
                    )
            _load_w("wq")
            _load_w("wql")
            wo_sb = consts.tile([128, HL, D], F16, name="wo")
            if loop_n > 1:
                nc.sync.dma_start(wo_sb[:], wo_v)
            sv_sb = consts.tile([1, B * DHL], F16, name="sv")
            nc.sync.dma_start(sv_sb[:], sv_d.ap())
            # [128, 2, 16] so the k-tile dim strides 16B (dual-fp8 ldweights
            # requires an even, 16B-aligned plane step); only column 0 is used.
            ones8 = consts.tile([128, 2, 16], F8, name="ones8")
            nc.vector.memset(ones8[:], 1.0)
            ones_row = consts.tile([1, 128], F16, name="ones_row")
            nc.vector.memset(ones_row[:], 1.0)
            ones512 = consts.tile([1, 512], F16, name="ones512")
            nc.vector.memset(ones512[:], 1.0)
            swap_mask = [(i + 16) % 32 for i in range(32)]

            qt_sb = qtkt.tile([128, HL, BN], F16, name="qt")
            kt_sb = qtkt.tile([128, HL, BN], F16, name="kt")
            v8_sb = vres.tile([128, BN // 128, DHL], F8, name="v8")

            # ---- phase 1: projections + rope ----
            import contextlib
            loop_cm = tc.For_i(0, loop_n, 1) if loop_n > 1 else contextlib.nullcontext()
            with loop_cm:
              for blk in range(NBLK):
                  b = blk // (NBLK // B)
                  c0 = (blk % (NBLK // B)) * 512
                  xblk = xin.tile([128, DCH, 512], F8, name="xblk")
                  for dq in range(4):
                      nc.sync.dma_start(
                          xblk[:, dq * 4:(dq + 1) * 4, :],
                          x8_v[:, dq * 4:(dq + 1) * 4, blk * 512:(blk + 1) * 512],
                      )
                  rblk = ropein.tile([128, 4, 512], F16, name="rblk")
                  nc.sync.dma_start(
                      rblk[:], rope_v[4 * b:4 * b + 4, :, c0:c0 + 512].rearrange("r p n -> p r n")
                  )
                  if blk == 0:
                      _load_w("wk")
                      _load_w("wkl")
                      _load_w("wv")

                  for wname, dst_sb, ra, rb_ in (
                      ("wq", qt_sb, 0, 1),
                      ("wk", kt_sb, 2, 3),
                  ):
                      for j in range(HL):
                          ps = psa.tile([128, 512], F32, name="pp")
                          for wn in (wname, wname + "l"):
                              for dc in range(DCH // 2):
                                  nc.tensor.matmul(
                                      ps[:],
                                      w_sb[wn][:, 2 * dc:2 * dc + 2, j * 128:(j + 1) * 128],
                                      xblk[:, 2 * dc:2 * dc + 2, :],
                                      start=(wn == wname and dc == 0),
                                      stop=(wn != wname and dc == DCH // 2 - 1),
                                      perf_mode=DR,
                                  )
                          raw = tmps.tile([128, 512], F16, name="raw")
                          nc.scalar.copy(raw[:], ps[:])
                          t2 = tmps.tile([128, 512], F16, name="t2")
                          nc.vector.stream_shuffle(t2[:], raw[:], swap_mask)
                          nc.vector.tensor_mul(t2[:], t2[:], rblk[:, rb_, :])
                          nc.vector.tensor_mul(raw[:], raw[:], rblk[:, ra, :])
                          nc.vector.tensor_add(
                              dst_sb[:, j, blk * 512:(blk + 1) * 512], raw[:], t2[:]
                          )

                  for mc in range(4):
                      psv = psb.tile([128, DHL], F32, name="pb")
                      for dc in range(DCH // 2):
                          nc.tensor.matmul(
                              psv[:],
                              xblk[:, 2 * dc:2 * dc + 2, mc * 128:(mc + 1) * 128],
                              w_sb["wv"][:, 2 * dc:2 * dc + 2, :],
                              start=(dc == 0),
                              stop=(dc == DCH // 2 - 1),
                              perf_mode=DR,
                          )
                      nc.scalar.activation(
                          v8_sb[:, blk * 4 + mc, :], psv[:], AF.Copy, scale=1.0 / WS
                      )

              if loop_n == 1:
                  nc.sync.dma_start(wo_sb[:], wo_v)
              _PHASE_MARKS['end_phase1'] = int(nc.get_next_instruction_name()[2:])
              # ---- phase 2+3 per batch ----
              for b in range(B if not p1_only else 0):
                  ot_tiles = [otbuf.tile([128, N], F16, name="ot") for _ in range(HL)]
                  for nck in range(NCK):
                      nq0 = b * N + nck * 512
                      for j in range(HL):
                          ot = ot_tiles[j]
                          pts8 = []
                          for mc2 in range(MCH // 2):
                              sp = psa.tile([128, 2, 512], F32, name="pp")
                              for half in range(2):
                                  mc = 2 * mc2 + half
                                  nc.tensor.matmul(
                                      sp[:, half, :],
                                      kt_sb[:, j, b * N + mc * 128:b * N + (mc + 1) * 128],
                                      qt_sb[:, j, nq0:nq0 + 512],
                                      start=True,
                                      stop=True,
                                  )
                              if p2_mode == "scores":
                                  if mc2 == 0:
                                      nc.scalar.activation(
                                          ot[:, nck * 512:(nck + 1) * 512],
                                          sp[:, 0, :], AF.Copy)
                                  continue
                              pt = ptile.tile([128, 2, 512], F16, name="pt")
                              nc.scalar.activation(pt[:], sp[:], AF.Exp)
                              if p2_mode == "exp":
                                  if mc2 == 0:
                                      nc.vector.tensor_copy(
                                          ot[:, nck * 512:(nck + 1) * 512], pt[:, 0, :])
                                  continue
                              pt8 = p8tile.tile([128, 2, 512], F8, name="pt8")
                              nc.vector.tensor_scalar_sub(pt8[:], pt[:], 1.0)
                              pts8.append(pt8)
                          if p2_mode in ("scores", "exp"):
                              continue

                          # attn@V: PSUM starts with the sum_m v deposit (K=1
                          # matmul of host-computed sv), then 8 fp8 DoubleRow
                          # passes add P' V.
                          op = psb.tile([128, 512], F32, name="pb")
                          nc.tensor.matmul(
                              op[:],
                              sv_sb[0:1, (b * HL + j) * 128:(b * HL + j + 1) * 128],
                              ones512[:],
                              start=True,
                              stop=False,
                          )
                          for mc2 in range(MCH // 2):
                              nc.tensor.matmul(
                                  op[:],
                                  v8_sb[:, b * MCH + 2 * mc2:b * MCH + 2 * mc2 + 2,
                                        j * 128:(j + 1) * 128],
                                  pts8[mc2][:],
                                  start=False,
                                  stop=(mc2 == MCH // 2 - 1),
                                  perf_mode=DR,
                              )

                          if p2_mode == "attnv":
                              nc.vector.tensor_copy(
                                  ot[:, nck * 512:(nck + 1) * 512], op[:])
                              continue
                          # denominators: den = 2048 + colsum(P') on the PE
                          dps = psc.tile([1, 512], F32, name="pc")
                          for mc2 in range(MCH // 2):
                              nc.tensor.matmul(
                                  dps[:],
                                  ones8[:, :, 0:1],
                                  pts8[mc2][:],
                                  start=(mc2 == 0),
                                  stop=(mc2 == MCH // 2 - 1),
                                  perf_mode=DR,
                              )
                          den = smalls.tile([1, 512], F32, name="den")
                          nc.vector.tensor_scalar_add(den[:], dps[:], float(N))
                          rc32 = smalls.tile([1, 512], F32, name="rc32")
                          nc.vector.reciprocal(rc32[:], den[:])
                          rc16 = smalls.tile([1, 512], F16, name="rc16")
                          nc.vector.tensor_copy(rc16[:], rc32[:])
                          bps = psc.tile([128, 512], F32, name="pc")
                          nc.tensor.matmul(bps[:], ones_row[:], rc16[:], start=True, stop=True)
                          rbc = rbcp.tile([128, 512], F16, name="rbc")
                          nc.vector.tensor_copy(rbc[:], bps[:])
                          nc.vector.tensor_mul(ot[:, nck * 512:(nck + 1) * 512], op[:], rbc[:])

                      # output projection for the n-range this nck covers
                      for nck2 in range(4):
                          ncol = nck * 4 + nck2
                          cb = b * (N // 128) + ncol
                          for dcol in range(D // 512):
                              ops3 = psb.tile([128, 512], F32, name="pb")
                              for j in range(HL):
                                  nc.tensor.matmul(
                                      ops3[:],
                                      ot_tiles[j][:, ncol * 128:(ncol + 1) * 128],
                                      wo_sb[:, j, dcol * 512:(dcol + 1) * 512],
                                      start=(j == 0),
                                      stop=(j == HL - 1),
                                  )
                              ob = obuf.tile([128, 512], F16, name="ob")
                              if dcol % 2 == 0:
                                  nc.scalar.copy(ob[:], ops3[:])
                              else:
                                  nc.vector.tensor_copy(ob[:], ops3[:])
                              nc.sync.dma_start(
                                  out_v[cb, :, dcol * 512:(dcol + 1) * 512], ob[:]
                              )
                  _PHASE_MARKS[f'end_b{b}'] = int(nc.get_next_instruction_name()[2:])
              _PHASE_MARKS['end'] = int(nc.get_next_instruction_name()[2:])
    nc.compile()
    return nc


# Permutation of the Dh dim: rotation-pair p = (2p, 2p+1) goes to partitions
# (qd*32 + j, qd*32 + 16 + j) with qd = p // 16, j = p % 16, so the
# real<->imag partner swap is a rotate-by-16 within each 32-partition quadrant
# (expressible as a DVE stream_shuffle).
_PERM = np.empty(Dh, dtype=np.int64)
_PAIR = np.empty(Dh, dtype=np.int64)   # rotation-pair index feeding each partition
_SGN = np.empty(Dh, dtype=np.float64)  # sign of the ri factor at each partition
for _qd in range(4):
    for _j in range(16):
        _p = _qd * 16 + _j
        _PERM[_qd * 32 + _j] = 2 * _p
        _PERM[_qd * 32 + 16 + _j] = 2 * _p + 1
        _PAIR[_qd * 32 + _j] = _p
        _PAIR[_qd * 32 + 16 + _j] = _p
        _SGN[_qd * 32 + _j] = -1.0
        _SGN[_qd * 32 + 16 + _j] = 1.0


def _prep_inputs(x, q_rope, k_rope, Wq, Wk, Wv, Wo):
    import ml_dtypes

    NF8 = ml_dtypes.float8_e4m3
    xt = x.reshape(BN, D).T.astype(np.float32)
    x8 = np.ascontiguousarray(xt.astype(NF8))

    # rope factor tensors: per batch [qrA, qrB, krA, krB], each [128, N].
    # Both Dh^-0.5 score scales fold into the q factors; both q and k factors
    # additionally carry 1/WS to undo the fp8 weight scaling.
    s = float(Dh) ** -1.0
    ropes = []
    for b in range(B):
        for r, scale in ((q_rope[b], s / WS), (k_rope[b], 1.0 / WS)):
            rr = r[:, 0::2].T * scale   # [64, N], indexed by rotation pair
            ri = r[:, 1::2].T * scale
            ropes.append(rr[_PAIR])                  # A: rr at both partners
            ropes.append(ri[_PAIR] * _SGN[:, None])  # B: -ri at real, +ri at imag
    rope_all = np.ascontiguousarray(np.stack(ropes).astype(np.float16))

    sx = x.sum(axis=1).astype(np.float32)  # [B, D]

    in_maps = []
    for c in range(N_CORES):
        heads = range(HL * c, HL * (c + 1))
        wq_c = np.concatenate(
            [Wq[:, h * Dh:(h + 1) * Dh][:, _PERM] for h in heads], axis=1
        ).astype(np.float32) * WS
        wk_c = np.concatenate(
            [Wk[:, h * Dh:(h + 1) * Dh][:, _PERM] for h in heads], axis=1
        ).astype(np.float32) * WS
        wv_c = np.concatenate(
            [Wv[:, h * Dh:(h + 1) * Dh] for h in heads], axis=1
        ).astype(np.float32)
        wq8 = wq_c.astype(NF8)
        wq8l = (wq_c - wq8.astype(np.float32)).astype(NF8)
        wk8 = wk_c.astype(NF8)
        wk8l = (wk_c - wk8.astype(np.float32)).astype(NF8)
        wo_c = np.concatenate(
            [Wo[h * Dh:(h + 1) * Dh, :] for h in heads], axis=0
        ).astype(np.float16)
        sv_c = (sx @ wv_c).reshape(1, B * DHL).astype(np.float16)  # [1, B*DHL]
        in_maps.append(
            {
                "x8": x8,
                "wq": np.ascontiguousarray(wq8),
                "wql": np.ascontiguousarray(wq8l),
                "wk": np.ascontiguousarray(wk8),
                "wkl": np.ascontiguousarray(wk8l),
                "wv": np.ascontiguousarray((wv_c * WS).astype(NF8)),
                "wo": np.ascontiguousarray(wo_c),
                "sv": sv_c,
                "rope": rope_all,
            }
        )
    return in_maps


def kernel(x, q_rope, k_rope, Wq, Wk, Wv, Wo, bo, **run_kwargs):
    if "nc" not in _CACHE:
        _CACHE["nc"] = _build_nc()
    nc = _CACHE["nc"]

    in_maps = _prep_inputs(x, q_rope, k_rope, Wq, Wk, Wv, Wo)
    res = run_bass_kernel_spmd(nc, in_maps, core_ids=list(range(N_CORES)), **run_kwargs)

    total = np.zeros((BN, D), dtype=np.float32)
    for c in range(N_CORES):
        total += res.results[c]["out"].astype(np.float32)
    total += bo.astype(np.float32)[None, :]
    out = total.reshape(B, N, D)
    _CACHE["last_res"] = res
    return out
